# revision 24
# baseline (speedup 1.0000x reference)
"""CopyGenerator on 8 TRN2 NeuronCores.

Strategy: tensor-parallel split of the 50257-wide generator vocab across the
8 cores (6400 padded columns each).  Per core:
  - W shard and hidden are quantized to fp8-e4m3 on host (W with one global
    scale, hidden with per-row scales) and fed to DoubleRow fp8 matmuls
    (2 k-subtiles per instruction -> 2x bf16 FLOPs on TRN2),
  - the combined dequant scale (s_row * s_W) is folded into the Exp
    activation's per-partition `scale` operand, so logits never materialize;
    a constant bias of -ln(32) keeps exp values inside fp8-e4m3 range,
  - exp results are stored fp8 in SBUF (25.6 KB/partition for 4 row tiles),
    with accum_out giving row partial sums,
  - softmax partial denominators are all-gathered once per FOUR row tiles
    ([128, 4] f32 - tiny, overlapped with the next group's matmuls),
  - exp is rescaled by (1 - p_copy)/denom on the DVE (fp8 in, bf16 out)
    and the output shard is written bf16 (host upcasts to f32),
  - copy gate z = h @ W_copy.T runs in bf16, copy-attention path in exact
    fp32, both redundantly on every core.  Gate/copy-path instructions are
    emitted between row tiles 1..3 so their late-arriving inputs never stall
    the in-order PE stream, and hidden/W DMAs are ordered so the first row
    tile's matmuls can start ~4us into the kernel.
PAD column and vocab-padding columns are handled by zeroing those W rows on
the host (=> logit 0, exp 1/32) and subtracting mcount/32 from the partial
denominator; the host zeroes the PAD output column.

kernel(**inputs) takes the full unsharded inputs and returns the full
[2048, 50321] float32 output.
"""

import os
import sys

for _p in ("/opt/trn_rl_repo", "/opt/trn_rl_repo/concourse"):
    if _p not in sys.path:
        sys.path.insert(0, _p)

from contextlib import ExitStack

import ml_dtypes
import numpy as np

import concourse.bass as bass
import concourse.mybir as mybir
import concourse.tile as tile
from concourse import bacc
from concourse.bass_utils import run_bass_kernel_spmd

# ---- problem constants (hardcoded per the self-contained-kernel contract) ----
N, D = 2048, 1024                 # tlen*batch rows, hidden dim
TLEN, BATCH, SLEN, CVOCAB = 64, 32, 128, 64
VOCAB = 50257
PAD_IDX = 0
NCORES = 8
VS = 6400                         # per-core padded vocab shard width
VPAD = VS * NCORES                # 51200
DT = D // 128                     # 8 contraction subtiles of 128
QT = DT // 2                      # 4 DoubleRow pairs
NT = N // 128                     # 16 row tiles
GRP = 4                           # row tiles per denominator collective
NG = NT // GRP
CH = [(0, 1536), (1536, 1536), (3072, 1536), (4608, 1536),
      (6144, 256)]                               # (offset, width) exp chunks
NCH = len(CH)
E4M3_MAX = 240.0
ATT = 32.0                        # exp attenuation keeping fp8 in range
BIAS = -float(np.log(ATT))
OSCALE = 4096.0                   # fp8 output scale (host divides it out)

E4 = ml_dtypes.float8_e4m3
BF16 = ml_dtypes.bfloat16
F32 = mybir.dt.float32
BF16_T = mybir.dt.bfloat16
FP8_T = mybir.dt.float8e4
DR = mybir.MatmulPerfMode.DoubleRow
DRS = mybir.MatmulPerfMode.DoubleRowSwInterleave

LAST_RESULTS = None               # BassKernelResults of the most recent run
_NC_CACHE = {}


def _build(bc_val: float, use_bgen: bool):
    nc = bacc.Bacc("TRN2", target_bir_lowering=False, debug=False,
                   num_devices=NCORES)

    wt = nc.dram_tensor("wt", [128, 2, QT, VS], FP8_T, kind="ExternalInput").ap()
    htq = nc.dram_tensor("htq", [128, NT, QT, 256], FP8_T,
                         kind="ExternalInput").ap()
    htb = nc.dram_tensor("htb", [128, DT, N], BF16_T, kind="ExternalInput").ap()
    attn_r = nc.dram_tensor("attn_r", [128, BATCH, TLEN], F32,
                            kind="ExternalInput").ap()
    smap = nc.dram_tensor("smap", [128, BATCH, CVOCAB], F32,
                          kind="ExternalInput").ap()
    wc = nc.dram_tensor("wc", [128, DT], BF16_T, kind="ExternalInput").ap()
    scol_d = nc.dram_tensor("scol", [128, NT], F32, kind="ExternalInput").ap()
    mneg = nc.dram_tensor("mneg", [1, 1], F32, kind="ExternalInput").ap()
    if use_bgen:
        bg = nc.dram_tensor("bg", [1, VS], BF16_T, kind="ExternalInput").ap()
    out_main = nc.dram_tensor("out_main", [N, VS], FP8_T,
                              kind="ExternalOutput").ap()
    out_copy = nc.dram_tensor("out_copy", [N, CVOCAB], F32,
                              kind="ExternalOutput").ap()

    with tile.TileContext(nc) as tc, ExitStack() as ctx:
        singles = ctx.enter_context(tc.tile_pool(name="singles", bufs=1))
        dram = ctx.enter_context(tc.tile_pool(name="dram", bufs=1, space="DRAM"))

        # ---- resident inputs; order = DMA issue order (first needed first,
        # tiny consumers of the first exps before the bulk weights) ----
        htq_sb = singles.tile([128, NT, QT, 256], FP8_T)
        nc.sync.dma_start(out=htq_sb[:, 0, :, :], in_=htq[:, 0, :, :])
        wt_sb = singles.tile([128, 2, QT, VS], FP8_T)
        nc.sync.dma_start(out=wt_sb[:, :, :, 0:512], in_=wt[:, :, :, 0:512])
        scol_sb = singles.tile([128, NT], F32)
        nc.sync.dma_start(out=scol_sb, in_=scol_d)
        wc_sb = singles.tile([128, DT], BF16_T)
        nc.sync.dma_start(out=wc_sb, in_=wc)
        mneg_sb = singles.tile([128, 1], F32)
        nc.gpsimd.dma_start(out=mneg_sb, in_=mneg.to_broadcast((128, 1)))
        bias_sb = singles.tile([128, 1], F32)
        nc.vector.memset(bias_sb, BIAS)
        for c0 in range(512, 2048, 512):
            nc.sync.dma_start(out=wt_sb[:, :, :, c0:c0 + 512],
                              in_=wt[:, :, :, c0:c0 + 512])
        nc.sync.dma_start(out=htq_sb[:, 1, :, :], in_=htq[:, 1, :, :])
        for c0 in range(2048, VS, 1024):
            cw = min(1024, VS - c0)
            nc.sync.dma_start(out=wt_sb[:, :, :, c0:c0 + cw],
                              in_=wt[:, :, :, c0:c0 + cw])
        for j in (2, 3):
            nc.sync.dma_start(out=htq_sb[:, j, :, :], in_=htq[:, j, :, :])
        htb_sb = singles.tile([128, DT, N], BF16_T)
        nc.sync.dma_start(out=htb_sb, in_=htb)
        attn_sb = singles.tile([128, BATCH, TLEN], F32)
        nc.sync.dma_start(out=attn_sb, in_=attn_r)
        sm_sb = singles.tile([128, BATCH, CVOCAB], F32)
        nc.sync.dma_start(out=sm_sb, in_=smap)
        nc.sync.dma_start(out=htq_sb[:, 4:NT, :, :], in_=htq[:, 4:NT, :, :])
        if use_bgen:
            bg_sb = singles.tile([1, VS], BF16_T)
            nc.sync.dma_start(out=bg_sb, in_=bg)
            ones_sb = singles.tile([1, N], BF16_T)
            nc.vector.memset(ones_sb, 1.0)

        zcol = singles.tile([128, NT], F32)
        ompcol = singles.tile([128, NT], F32)   # 1 - p_copy = sigmoid(-z - bc)

        cps = ctx.enter_context(tc.tile_pool(name="cps", bufs=1))
        ocp = ctx.enter_context(tc.tile_pool(name="ocp", bufs=2))
        expp = ctx.enter_context(tc.tile_pool(name="expp", bufs=8))
        accp = ctx.enter_context(tc.tile_pool(name="accp", bufs=4))
        small = ctx.enter_context(tc.tile_pool(name="small", bufs=4))
        ostp = ctx.enter_context(tc.tile_pool(name="ostp", bufs=2))
        ps_cp = ctx.enter_context(
            tc.tile_pool(name="ps_cp", bufs=1, space="PSUM"))
        ps_256 = ctx.enter_context(
            tc.tile_pool(name="ps_256", bufs=1, space="PSUM"))
        ps_main = ctx.enter_context(
            tc.tile_pool(name="ps_main", bufs=2, space="PSUM"))

        def emit_gate():
            # ---- copy-gate z = hidden @ W_copy.T  (M=1 bf16 matmuls) ----
            z_sb = cps.tile([1, N], F32)
            for q in range(N // 512):
                zp = ps_cp.tile([1, 512], F32, tag="cp",
                                padded_shape=[TLEN, 8 * CVOCAB])
                for d in range(DT):
                    nc.tensor.matmul(
                        zp,
                        lhsT=wc_sb[:, d:d + 1],
                        rhs=htb_sb[:, d, q * 512:(q + 1) * 512],
                        start=(d == 0), stop=(d == DT - 1),
                    )
                nc.scalar.copy(out=z_sb[:, q * 512:(q + 1) * 512], in_=zp)
            zdram = dram.tile([N], F32)
            nc.sync.dma_start(out=zdram.rearrange("(a n) -> a n", a=1), in_=z_sb)
            # per-row-tile column layout [128, 16] and per-(t,b) layout [64, 32]
            nc.scalar.dma_start(out=zcol,
                                in_=zdram.rearrange("(j p) -> p j", p=128))
            zbt = cps.tile([TLEN, BATCH], F32)
            nc.scalar.dma_start(out=zbt,
                                in_=zdram.rearrange("(t b) -> t b", b=BATCH))
            nc.scalar.activation(ompcol, zcol,
                                 mybir.ActivationFunctionType.Sigmoid,
                                 bias=-bc_val, scale=-1.0)
            pcbt = cps.tile([TLEN, BATCH], F32)  # p_copy = sigmoid(z + bc)
            nc.scalar.activation(pcbt, zbt,
                                 mybir.ActivationFunctionType.Sigmoid,
                                 bias=bc_val, scale=1.0)
            return pcbt

        def emit_copy_path(pcbt):
            # ---- copy path: per-batch [64t,128s] @ [128s,64c] x p_copy,
            # 8 batches share one PSUM bank / output DMA so the PE stream
            # never waits on single-batch drains ----
            oc3 = out_copy.rearrange("(t b) c -> t (b c)", b=BATCH)
            for g in range(BATCH // 8):
                cp = ps_cp.tile([TLEN, 8 * CVOCAB], F32, tag="cp")
                for bi in range(8):
                    b = g * 8 + bi
                    nc.tensor.matmul(
                        cp[:, bi * CVOCAB:(bi + 1) * CVOCAB],
                        lhsT=attn_sb[:, b, :],
                        rhs=sm_sb[:, b, :],
                        start=True, stop=True,
                    )
                oc = ocp.tile([TLEN, 8 * CVOCAB], F32, tag="oc")
                for bi in range(8):
                    b = g * 8 + bi
                    nc.vector.tensor_scalar_mul(
                        oc[:, bi * CVOCAB:(bi + 1) * CVOCAB],
                        cp[:, bi * CVOCAB:(bi + 1) * CVOCAB],
                        pcbt[:, b:b + 1])
                nc.sync.dma_start(
                    out=oc3[:, g * 8 * CVOCAB:(g + 1) * 8 * CVOCAB], in_=oc)

        # ---- main loop: one denominator collective per 4 row tiles ----
        pcbt = None
        exp_tiles = {}
        for g0, gsz in [(0, 4), (4, 4), (8, 6), (14, 2)]:
            ccst = small.tile([128, gsz], F32, tag=f"ccst{gsz}")
            for t in range(gsz):
                j = g0 + t
                exp_sb = expp.tile([128, VS], FP8_T, tag="exp")
                exp_tiles[j] = exp_sb
                acc7 = accp.tile([128, NCH], F32, tag="acc7")
                for ch in range(NCH):
                    c0, cw = CH[ch]
                    if cw == 256:
                        psm = ps_256.tile([128, 256], F32, tag="p256")
                    else:
                        psm = ps_main.tile([128, cw], F32, tag="psm",
                                           padded_shape=[128, 1536])
                    for q in range(QT):
                        for h0 in range(0, cw, 512):
                            hw = min(512, cw - h0)
                            nc.tensor.matmul(
                                psm[:, h0:h0 + hw],
                                lhsT=htq_sb[:, j, q, :],
                                rhs=wt_sb[:, :, q, c0 + h0:c0 + h0 + hw],
                                start=(q == 0),
                                stop=(q == QT - 1) and not use_bgen,
                                perf_mode=DRS,
                                skip_group_check=True,
                            )
                    if use_bgen:
                        for h0 in range(0, cw, 512):
                            hw = min(512, cw - h0)
                            nc.tensor.matmul(
                                psm[:, h0:h0 + hw],
                                lhsT=ones_sb[:, j * 128:(j + 1) * 128],
                                rhs=bg_sb[:, c0 + h0:c0 + h0 + hw],
                                start=False, stop=True,
                                skip_group_check=True,
                            )
                    nc.scalar.activation(exp_sb[:, c0:c0 + cw],
                                         psm,
                                         mybir.ActivationFunctionType.Exp,
                                         bias=bias_sb,
                                         scale=scol_sb[:, j:j + 1],
                                         accum_out=acc7[:, ch:ch + 1])
                accsum = small.tile([128, 1], F32, tag="accsum")
                nc.vector.reduce_sum(accsum, acc7, axis=mybir.AxisListType.X)
                nc.vector.tensor_scalar_add(ccst[:, t:t + 1], accsum, mneg_sb)
                if j == 2:
                    pcbt = emit_gate()
                elif j == 4:
                    emit_copy_path(pcbt)
            ccin = dram.tile([128, gsz], F32, tag=f"ccin{gsz}", bufs=2)
            nc.sync.dma_start(out=ccin, in_=ccst)
            ccout = dram.tile([NCORES * 128 * gsz], F32, tag=f"ccout{gsz}",
                              bufs=2)
            nc.gpsimd.collective_compute(
                "AllGather", mybir.AluOpType.bypass,
                replica_groups=[list(range(NCORES))],
                ins=[ccin.opt()], outs=[ccout.opt()],
            )
            parts = small.tile([128, gsz, NCORES], F32, tag=f"parts{gsz}")
            nc.sync.dma_start(
                out=parts,
                in_=ccout.rearrange("(r p t) -> p t r", p=128, t=gsz))
            last = g0 + gsz == NT
            fss = {}
            if last:
                for t in range(gsz):
                    j = g0 + t
                    denom = small.tile([128, 1], F32, tag="denom")
                    nc.vector.reduce_sum(denom, parts[:, t, :],
                                         axis=mybir.AxisListType.X)
                    rden = small.tile([128, 1], F32, tag="rden")
                    nc.vector.reciprocal(rden, denom)
                    fs = small.tile([128, 1], F32, tag=f"fs{t}")
                    nc.vector.tensor_scalar(fs, rden, ompcol[:, j:j + 1],
                                            OSCALE, mybir.AluOpType.mult,
                                            mybir.AluOpType.mult)
                    fss[j] = fs
            for t in range(gsz):
                j = g0 + t
                n0 = j * 128
                if last:
                    fs = fss[j]
                else:
                    denom = small.tile([128, 1], F32, tag="denom")
                    nc.vector.reduce_sum(denom, parts[:, t, :],
                                         axis=mybir.AxisListType.X)
                    rden = small.tile([128, 1], F32, tag="rden")
                    nc.vector.reciprocal(rden, denom)
                    fs = small.tile([128, 1], F32, tag="fs")
                    nc.vector.tensor_scalar(fs, rden, ompcol[:, j:j + 1],
                                            OSCALE, mybir.AluOpType.mult,
                                            mybir.AluOpType.mult)
                ost = ostp.tile([128, VS], FP8_T, tag="ost")
                if last:
                    # drain the final group on both element-wise engines
                    # (2/3 DVE, 1/3 Act Copy) so the tail is DMA-bound
                    HV = 4352
                    nc.vector.tensor_scalar_mul(ost[:, 0:HV],
                                                exp_tiles[j][:, 0:HV], fs)
                    nc.sync.dma_start(out=out_main[n0:n0 + 128, 0:HV],
                                      in_=ost[:, 0:HV])
                    nc.scalar.activation(ost[:, HV:VS], exp_tiles[j][:, HV:VS],
                                         mybir.ActivationFunctionType.Copy,
                                         scale=fs)
                    nc.sync.dma_start(out=out_main[n0:n0 + 128, HV:VS],
                                      in_=ost[:, HV:VS])
                else:
                    nc.vector.tensor_scalar_mul(ost, exp_tiles[j], fs)
                    nc.sync.dma_start(out=out_main[n0:n0 + 128, :], in_=ost)

    nc.compile()
    return nc


def _get_nc(bc_val: float, use_bgen: bool):
    key = (bc_val, use_bgen)
    if key not in _NC_CACHE:
        _NC_CACHE[key] = _build(bc_val, use_bgen)
    return _NC_CACHE[key]


def kernel(hidden, attn, src_map, W_gen, b_gen, W_copy, b_copy):
    global LAST_RESULTS
    hidden = np.asarray(hidden, dtype=np.float32)
    attn = np.asarray(attn, dtype=np.float32)
    src_map = np.asarray(src_map, dtype=np.float32)
    W_gen = np.asarray(W_gen, dtype=np.float32)
    b_gen = np.asarray(b_gen, dtype=np.float32)
    W_copy = np.asarray(W_copy, dtype=np.float32)
    b_copy = np.asarray(b_copy, dtype=np.float32)

    use_bgen = bool(np.any(b_gen))
    bc_val = float(b_copy.reshape(-1)[0])
    nc = _get_nc(bc_val, use_bgen)

    # per-row fp8 scales for hidden; one global scale for W
    s_n = np.abs(hidden).max(axis=1, keepdims=True) / E4M3_MAX   # [N, 1]
    s_n = np.maximum(s_n, 1e-30)
    hq = (hidden / s_n).astype(E4)
    s_w = max(float(np.abs(W_gen).max()) / E4M3_MAX, 1e-30)

    # hq tiled for DoubleRowSwInterleave, row-tile-major; per (j, q) the
    # [128, 256] stationary block holds (A,B) k-subtile pairs interleaved
    # per column with columns reversed: swi[p, 2k+i] = hq[j*128+127-k,
    # q*256 + i*128 + p]
    htq = hq.reshape(NT, 128, QT, 2, 128).transpose(4, 0, 2, 1, 3)
    htq = np.ascontiguousarray(htq[:, :, :, ::-1, :]).reshape(
        128, NT, QT, 256)
    # bf16 hidden for the copy gate: htb[p, d, n] = h[n, d*128 + p]
    htb = np.ascontiguousarray(
        hidden.reshape(N, DT, 128).transpose(2, 1, 0)).astype(BF16)

    # combined dequant scale per row, tiled [128, NT]
    scol = np.ascontiguousarray(
        (s_n[:, 0] * s_w).astype(np.float32).reshape(NT, 128).T)

    # padded W with masked rows zeroed (PAD row + vocab padding)
    Wp = np.zeros((VPAD, D), dtype=np.float32)
    Wp[:VOCAB] = W_gen
    Wp[PAD_IDX] = 0.0
    Wq = (Wp / s_w).astype(E4)
    Wq[PAD_IDX] = 0
    Wq[VOCAB:] = 0
    if use_bgen:
        bgp = np.zeros((VPAD,), dtype=np.float32)
        bgp[:VOCAB] = b_gen
        bgp[PAD_IDX] = 0.0

    # attn rearranged to [s, b, t] f32; smap [s, b, c] f32
    attn_r = np.ascontiguousarray(
        attn.reshape(TLEN, BATCH, SLEN).transpose(2, 1, 0))
    smap = np.ascontiguousarray(src_map.astype(np.float32))
    wc = np.ascontiguousarray(W_copy[0].reshape(DT, 128).T).astype(BF16)

    masked = np.zeros(VPAD, dtype=bool)
    masked[PAD_IDX] = True
    masked[VOCAB:] = True

    in_maps = []
    for c in range(NCORES):
        shard = Wq[c * VS:(c + 1) * VS]            # [VS, D] fp8
        wt_c = np.ascontiguousarray(
            shard.reshape(VS, QT, 2, 128).transpose(3, 2, 1, 0))
        mcount = int(masked[c * VS:(c + 1) * VS].sum())
        m = {
            "wt": wt_c,
            "htq": htq,
            "htb": htb,
            "attn_r": attn_r,
            "smap": smap,
            "wc": wc,
            "scol": scol,
            "mneg": np.array([[-float(mcount) / ATT]], dtype=np.float32),
        }
        if use_bgen:
            m["bg"] = bgp[c * VS:(c + 1) * VS].reshape(1, VS).astype(BF16)
        in_maps.append(m)

    res = run_bass_kernel_spmd(nc, in_maps, core_ids=list(range(NCORES)))
    LAST_RESULTS = res

    out = np.empty((N, VOCAB + CVOCAB), dtype=np.float32)
    for c in range(NCORES):
        lo = c * VS
        hi = min(lo + VS, VOCAB)
        if hi > lo:
            out[:, lo:hi] = res.results[c]["out_main"][:, :hi - lo].astype(
                np.float32)
    out[:, :VOCAB] *= np.float32(1.0 / OSCALE)
    out[:, PAD_IDX] = 0.0
    out[:, VOCAB:] = res.results[0]["out_copy"]
    return out


if __name__ == "__main__":
    # build-only smoke test
    nc = _get_nc(0.0, False)
    print("build OK:", nc)


# revision 25
# speedup vs baseline: 1.0244x; 1.0244x over previous
"""CopyGenerator on 8 TRN2 NeuronCores.

Strategy: tensor-parallel split of the 50257-wide generator vocab across the
8 cores (6400 padded columns each).  Per core:
  - W shard and hidden are quantized to fp8-e4m3 on host (W with one global
    scale, hidden with per-row scales) and fed to DoubleRow fp8 matmuls
    (2 k-subtiles per instruction -> 2x bf16 FLOPs on TRN2),
  - the combined dequant scale (s_row * s_W) is folded into the Exp
    activation's per-partition `scale` operand, so logits never materialize;
    a constant bias of -ln(32) keeps exp values inside fp8-e4m3 range,
  - exp results are stored fp8 in SBUF (25.6 KB/partition for 4 row tiles),
    with accum_out giving row partial sums,
  - softmax partial denominators are all-gathered once per FOUR row tiles
    ([128, 4] f32 - tiny, overlapped with the next group's matmuls),
  - exp is rescaled by (1 - p_copy)/denom on the DVE (fp8 in, bf16 out)
    and the output shard is written bf16 (host upcasts to f32),
  - copy gate z = h @ W_copy.T runs in bf16, copy-attention path in exact
    fp32, both redundantly on every core.  Gate/copy-path instructions are
    emitted between row tiles 1..3 so their late-arriving inputs never stall
    the in-order PE stream, and hidden/W DMAs are ordered so the first row
    tile's matmuls can start ~4us into the kernel.
PAD column and vocab-padding columns are handled by zeroing those W rows on
the host (=> logit 0, exp 1/32) and subtracting mcount/32 from the partial
denominator; the host zeroes the PAD output column.

kernel(**inputs) takes the full unsharded inputs and returns the full
[2048, 50321] float32 output.
"""

import os
import sys

for _p in ("/opt/trn_rl_repo", "/opt/trn_rl_repo/concourse"):
    if _p not in sys.path:
        sys.path.insert(0, _p)

from contextlib import ExitStack

import ml_dtypes
import numpy as np

import concourse.bass as bass
import concourse.mybir as mybir
import concourse.tile as tile
from concourse import bacc
from concourse.bass_utils import run_bass_kernel_spmd

# ---- problem constants (hardcoded per the self-contained-kernel contract) ----
N, D = 2048, 1024                 # tlen*batch rows, hidden dim
TLEN, BATCH, SLEN, CVOCAB = 64, 32, 128, 64
VOCAB = 50257
PAD_IDX = 0
NCORES = 8
VS = 6400                         # per-core padded vocab shard width
VPAD = VS * NCORES                # 51200
DT = D // 128                     # 8 contraction subtiles of 128
QT = DT // 2                      # 4 DoubleRow pairs
NT = N // 128                     # 16 row tiles
GRP = 4                           # row tiles per denominator collective
NG = NT // GRP
CH = [(0, 1536), (1536, 1536), (3072, 1536), (4608, 1536),
      (6144, 256)]                               # (offset, width) exp chunks
NCH = len(CH)
E4M3_MAX = 240.0
ATT = 32.0                        # exp attenuation keeping fp8 in range
BIAS = -float(np.log(ATT))
OSCALE = 4096.0                   # fp8 output scale (host divides it out)

E4 = ml_dtypes.float8_e4m3
BF16 = ml_dtypes.bfloat16
F32 = mybir.dt.float32
BF16_T = mybir.dt.bfloat16
FP8_T = mybir.dt.float8e4
DR = mybir.MatmulPerfMode.DoubleRow
DRS = mybir.MatmulPerfMode.DoubleRowSwInterleave

LAST_RESULTS = None               # BassKernelResults of the most recent run
_NC_CACHE = {}


def _build(bc_val: float, use_bgen: bool):
    nc = bacc.Bacc("TRN2", target_bir_lowering=False, debug=False,
                   num_devices=NCORES)

    wt = nc.dram_tensor("wt", [128, 2, QT, VS], FP8_T, kind="ExternalInput").ap()
    htq = nc.dram_tensor("htq", [128, NT, QT, 256], FP8_T,
                         kind="ExternalInput").ap()
    htb = nc.dram_tensor("htb", [128, DT, N], BF16_T, kind="ExternalInput").ap()
    attn_r = nc.dram_tensor("attn_r", [128, BATCH, TLEN], F32,
                            kind="ExternalInput").ap()
    smap = nc.dram_tensor("smap", [128, BATCH, CVOCAB], F32,
                          kind="ExternalInput").ap()
    wc = nc.dram_tensor("wc", [128, DT], BF16_T, kind="ExternalInput").ap()
    scol_d = nc.dram_tensor("scol", [128, NT], F32, kind="ExternalInput").ap()
    mneg = nc.dram_tensor("mneg", [1, 1], F32, kind="ExternalInput").ap()
    if use_bgen:
        bg = nc.dram_tensor("bg", [1, VS], BF16_T, kind="ExternalInput").ap()
    out_main = nc.dram_tensor("out_main", [N, VS], FP8_T,
                              kind="ExternalOutput").ap()
    out_copy = nc.dram_tensor("out_copy", [N, CVOCAB], F32,
                              kind="ExternalOutput").ap()

    with tile.TileContext(nc) as tc, ExitStack() as ctx:
        singles = ctx.enter_context(tc.tile_pool(name="singles", bufs=1))
        dram = ctx.enter_context(tc.tile_pool(name="dram", bufs=1, space="DRAM"))

        # ---- resident inputs; order = DMA issue order (first needed first,
        # tiny consumers of the first exps before the bulk weights) ----
        htq_sb = singles.tile([128, NT, QT, 256], FP8_T)
        nc.sync.dma_start(out=htq_sb[:, 0, :, :], in_=htq[:, 0, :, :])
        wt_sb = singles.tile([128, 2, QT, VS], FP8_T)
        nc.sync.dma_start(out=wt_sb[:, :, :, 0:512], in_=wt[:, :, :, 0:512])
        scol_sb = singles.tile([128, NT], F32)
        nc.sync.dma_start(out=scol_sb, in_=scol_d)
        wc_sb = singles.tile([128, DT], BF16_T)
        nc.sync.dma_start(out=wc_sb, in_=wc)
        mneg_sb = singles.tile([128, 1], F32)
        nc.gpsimd.dma_start(out=mneg_sb, in_=mneg.to_broadcast((128, 1)))
        bias_sb = singles.tile([128, 1], F32)
        nc.vector.memset(bias_sb, BIAS)
        for c0 in range(512, 2048, 512):
            nc.sync.dma_start(out=wt_sb[:, :, :, c0:c0 + 512],
                              in_=wt[:, :, :, c0:c0 + 512])
        nc.sync.dma_start(out=htq_sb[:, 1, :, :], in_=htq[:, 1, :, :])
        for c0 in range(2048, VS, 1024):
            cw = min(1024, VS - c0)
            nc.sync.dma_start(out=wt_sb[:, :, :, c0:c0 + cw],
                              in_=wt[:, :, :, c0:c0 + cw])
        for j in (2, 3):
            nc.sync.dma_start(out=htq_sb[:, j, :, :], in_=htq[:, j, :, :])
        htb_sb = singles.tile([128, DT, N], BF16_T)
        nc.sync.dma_start(out=htb_sb, in_=htb)
        attn_sb = singles.tile([128, BATCH, TLEN], F32)
        nc.sync.dma_start(out=attn_sb, in_=attn_r)
        sm_sb = singles.tile([128, BATCH, CVOCAB], F32)
        nc.sync.dma_start(out=sm_sb, in_=smap)
        nc.sync.dma_start(out=htq_sb[:, 4:NT, :, :], in_=htq[:, 4:NT, :, :])
        if use_bgen:
            bg_sb = singles.tile([1, VS], BF16_T)
            nc.sync.dma_start(out=bg_sb, in_=bg)
            ones_sb = singles.tile([1, N], BF16_T)
            nc.vector.memset(ones_sb, 1.0)

        zcol = singles.tile([128, NT], F32)
        ompcol = singles.tile([128, NT], F32)   # 1 - p_copy = sigmoid(-z - bc)

        cps = ctx.enter_context(tc.tile_pool(name="cps", bufs=1))
        ocp = ctx.enter_context(tc.tile_pool(name="ocp", bufs=2))
        expp = ctx.enter_context(tc.tile_pool(name="expp", bufs=8))
        accp = ctx.enter_context(tc.tile_pool(name="accp", bufs=4))
        small = ctx.enter_context(tc.tile_pool(name="small", bufs=4))
        ostp = ctx.enter_context(tc.tile_pool(name="ostp", bufs=2))
        ps_cp = ctx.enter_context(
            tc.tile_pool(name="ps_cp", bufs=1, space="PSUM"))
        ps_256 = ctx.enter_context(
            tc.tile_pool(name="ps_256", bufs=1, space="PSUM"))
        ps_main = ctx.enter_context(
            tc.tile_pool(name="ps_main", bufs=2, space="PSUM"))

        def emit_gate():
            # ---- copy-gate z = hidden @ W_copy.T  (M=1 bf16 matmuls) ----
            z_sb = cps.tile([1, N], F32)
            for q in range(N // 512):
                zp = ps_cp.tile([1, 512], F32, tag="cp",
                                padded_shape=[TLEN, 8 * CVOCAB])
                for d in range(DT):
                    nc.tensor.matmul(
                        zp,
                        lhsT=wc_sb[:, d:d + 1],
                        rhs=htb_sb[:, d, q * 512:(q + 1) * 512],
                        start=(d == 0), stop=(d == DT - 1),
                    )
                nc.scalar.copy(out=z_sb[:, q * 512:(q + 1) * 512], in_=zp)
            zdram = dram.tile([N], F32)
            nc.sync.dma_start(out=zdram.rearrange("(a n) -> a n", a=1), in_=z_sb)
            # per-row-tile column layout [128, 16] and per-(t,b) layout [64, 32]
            nc.scalar.dma_start(out=zcol,
                                in_=zdram.rearrange("(j p) -> p j", p=128))
            zbt = cps.tile([TLEN, BATCH], F32)
            nc.scalar.dma_start(out=zbt,
                                in_=zdram.rearrange("(t b) -> t b", b=BATCH))
            nc.scalar.activation(ompcol, zcol,
                                 mybir.ActivationFunctionType.Sigmoid,
                                 bias=-bc_val, scale=-1.0)
            pcbt = cps.tile([TLEN, BATCH], F32)  # p_copy = sigmoid(z + bc)
            nc.scalar.activation(pcbt, zbt,
                                 mybir.ActivationFunctionType.Sigmoid,
                                 bias=bc_val, scale=1.0)
            return pcbt

        def emit_copy_path(pcbt):
            # ---- copy path: per-batch [64t,128s] @ [128s,64c] x p_copy,
            # 8 batches share one PSUM bank / output DMA so the PE stream
            # never waits on single-batch drains ----
            oc3 = out_copy.rearrange("(t b) c -> t (b c)", b=BATCH)
            for g in range(BATCH // 8):
                cp = ps_cp.tile([TLEN, 8 * CVOCAB], F32, tag="cp")
                for bi in range(8):
                    b = g * 8 + bi
                    nc.tensor.matmul(
                        cp[:, bi * CVOCAB:(bi + 1) * CVOCAB],
                        lhsT=attn_sb[:, b, :],
                        rhs=sm_sb[:, b, :],
                        start=True, stop=True,
                    )
                oc = ocp.tile([TLEN, 8 * CVOCAB], F32, tag="oc")
                for bi in range(8):
                    b = g * 8 + bi
                    nc.vector.tensor_scalar_mul(
                        oc[:, bi * CVOCAB:(bi + 1) * CVOCAB],
                        cp[:, bi * CVOCAB:(bi + 1) * CVOCAB],
                        pcbt[:, b:b + 1])
                nc.sync.dma_start(
                    out=oc3[:, g * 8 * CVOCAB:(g + 1) * 8 * CVOCAB], in_=oc)

        # ---- main loop: one denominator collective per 4 row tiles ----
        pcbt = None
        exp_tiles = {}
        for g0, gsz in [(0, 2), (2, 4), (6, 4), (10, 4), (14, 2)]:
            ccst = small.tile([128, gsz], F32, tag=f"ccst{gsz}")
            for t in range(gsz):
                j = g0 + t
                exp_sb = expp.tile([128, VS], FP8_T, tag="exp")
                exp_tiles[j] = exp_sb
                acc7 = accp.tile([128, NCH], F32, tag="acc7")
                for ch in range(NCH):
                    c0, cw = CH[ch]
                    if cw == 256:
                        psm = ps_256.tile([128, 256], F32, tag="p256")
                    else:
                        psm = ps_main.tile([128, cw], F32, tag="psm",
                                           padded_shape=[128, 1536])
                    for h0 in range(0, cw, 512):
                        hw = min(512, cw - h0)
                        for q in range(QT):
                            nc.tensor.matmul(
                                psm[:, h0:h0 + hw],
                                lhsT=htq_sb[:, j, q, :],
                                rhs=wt_sb[:, :, q, c0 + h0:c0 + h0 + hw],
                                start=(q == 0),
                                stop=(q == QT - 1) and not use_bgen,
                                perf_mode=DRS,
                            )
                        if use_bgen:
                            nc.tensor.matmul(
                                psm[:, h0:h0 + hw],
                                lhsT=ones_sb[:, j * 128:(j + 1) * 128],
                                rhs=bg_sb[:, c0 + h0:c0 + h0 + hw],
                                start=False, stop=True,
                            )
                    nc.scalar.activation(exp_sb[:, c0:c0 + cw],
                                         psm,
                                         mybir.ActivationFunctionType.Exp,
                                         bias=bias_sb,
                                         scale=scol_sb[:, j:j + 1],
                                         accum_out=acc7[:, ch:ch + 1])
                accsum = small.tile([128, 1], F32, tag="accsum")
                nc.vector.reduce_sum(accsum, acc7, axis=mybir.AxisListType.X)
                nc.vector.tensor_scalar_add(ccst[:, t:t + 1], accsum, mneg_sb)
                if j == 2:
                    pcbt = emit_gate()
                elif j == 4:
                    emit_copy_path(pcbt)
            ccin = dram.tile([128, gsz], F32, tag=f"ccin{gsz}", bufs=2)
            nc.sync.dma_start(out=ccin, in_=ccst)
            ccout = dram.tile([NCORES * 128 * gsz], F32, tag=f"ccout{gsz}",
                              bufs=2)
            nc.gpsimd.collective_compute(
                "AllGather", mybir.AluOpType.bypass,
                replica_groups=[list(range(NCORES))],
                ins=[ccin.opt()], outs=[ccout.opt()],
            )
            parts = small.tile([128, gsz, NCORES], F32, tag=f"parts{gsz}")
            nc.sync.dma_start(
                out=parts,
                in_=ccout.rearrange("(r p t) -> p t r", p=128, t=gsz))
            last = g0 + gsz == NT
            fss = {}
            if last:
                for t in range(gsz):
                    j = g0 + t
                    denom = small.tile([128, 1], F32, tag="denom")
                    nc.vector.reduce_sum(denom, parts[:, t, :],
                                         axis=mybir.AxisListType.X)
                    rden = small.tile([128, 1], F32, tag="rden")
                    nc.vector.reciprocal(rden, denom)
                    fs = small.tile([128, 1], F32, tag=f"fs{t}")
                    nc.vector.tensor_scalar(fs, rden, ompcol[:, j:j + 1],
                                            OSCALE, mybir.AluOpType.mult,
                                            mybir.AluOpType.mult)
                    fss[j] = fs
            for t in range(gsz):
                j = g0 + t
                n0 = j * 128
                if last:
                    fs = fss[j]
                else:
                    denom = small.tile([128, 1], F32, tag="denom")
                    nc.vector.reduce_sum(denom, parts[:, t, :],
                                         axis=mybir.AxisListType.X)
                    rden = small.tile([128, 1], F32, tag="rden")
                    nc.vector.reciprocal(rden, denom)
                    fs = small.tile([128, 1], F32, tag="fs")
                    nc.vector.tensor_scalar(fs, rden, ompcol[:, j:j + 1],
                                            OSCALE, mybir.AluOpType.mult,
                                            mybir.AluOpType.mult)
                ost = ostp.tile([128, VS], FP8_T, tag="ost")
                if last:
                    # drain the final group on both element-wise engines
                    # (2/3 DVE, 1/3 Act Copy) so the tail is DMA-bound
                    HV = 4352
                    nc.vector.tensor_scalar_mul(ost[:, 0:HV],
                                                exp_tiles[j][:, 0:HV], fs)
                    nc.sync.dma_start(out=out_main[n0:n0 + 128, 0:HV],
                                      in_=ost[:, 0:HV])
                    nc.scalar.activation(ost[:, HV:VS], exp_tiles[j][:, HV:VS],
                                         mybir.ActivationFunctionType.Copy,
                                         scale=fs)
                    nc.sync.dma_start(out=out_main[n0:n0 + 128, HV:VS],
                                      in_=ost[:, HV:VS])
                else:
                    nc.vector.tensor_scalar_mul(ost, exp_tiles[j], fs)
                    nc.sync.dma_start(out=out_main[n0:n0 + 128, :], in_=ost)

    nc.compile()
    return nc


def _get_nc(bc_val: float, use_bgen: bool):
    key = (bc_val, use_bgen)
    if key not in _NC_CACHE:
        _NC_CACHE[key] = _build(bc_val, use_bgen)
    return _NC_CACHE[key]


def kernel(hidden, attn, src_map, W_gen, b_gen, W_copy, b_copy):
    global LAST_RESULTS
    hidden = np.asarray(hidden, dtype=np.float32)
    attn = np.asarray(attn, dtype=np.float32)
    src_map = np.asarray(src_map, dtype=np.float32)
    W_gen = np.asarray(W_gen, dtype=np.float32)
    b_gen = np.asarray(b_gen, dtype=np.float32)
    W_copy = np.asarray(W_copy, dtype=np.float32)
    b_copy = np.asarray(b_copy, dtype=np.float32)

    use_bgen = bool(np.any(b_gen))
    bc_val = float(b_copy.reshape(-1)[0])
    nc = _get_nc(bc_val, use_bgen)

    # per-row fp8 scales for hidden; one global scale for W
    s_n = np.abs(hidden).max(axis=1, keepdims=True) / E4M3_MAX   # [N, 1]
    s_n = np.maximum(s_n, 1e-30)
    hq = (hidden / s_n).astype(E4)
    s_w = max(float(np.abs(W_gen).max()) / E4M3_MAX, 1e-30)

    # hq tiled for DoubleRowSwInterleave, row-tile-major; per (j, q) the
    # [128, 256] stationary block holds (A,B) k-subtile pairs interleaved
    # per column with columns reversed: swi[p, 2k+i] = hq[j*128+127-k,
    # q*256 + i*128 + p]
    htq = hq.reshape(NT, 128, QT, 2, 128).transpose(4, 0, 2, 1, 3)
    htq = np.ascontiguousarray(htq[:, :, :, ::-1, :]).reshape(
        128, NT, QT, 256)
    # bf16 hidden for the copy gate: htb[p, d, n] = h[n, d*128 + p]
    htb = np.ascontiguousarray(
        hidden.reshape(N, DT, 128).transpose(2, 1, 0)).astype(BF16)

    # combined dequant scale per row, tiled [128, NT]
    scol = np.ascontiguousarray(
        (s_n[:, 0] * s_w).astype(np.float32).reshape(NT, 128).T)

    # padded W with masked rows zeroed (PAD row + vocab padding)
    Wp = np.zeros((VPAD, D), dtype=np.float32)
    Wp[:VOCAB] = W_gen
    Wp[PAD_IDX] = 0.0
    Wq = (Wp / s_w).astype(E4)
    Wq[PAD_IDX] = 0
    Wq[VOCAB:] = 0
    if use_bgen:
        bgp = np.zeros((VPAD,), dtype=np.float32)
        bgp[:VOCAB] = b_gen
        bgp[PAD_IDX] = 0.0

    # attn rearranged to [s, b, t] f32; smap [s, b, c] f32
    attn_r = np.ascontiguousarray(
        attn.reshape(TLEN, BATCH, SLEN).transpose(2, 1, 0))
    smap = np.ascontiguousarray(src_map.astype(np.float32))
    wc = np.ascontiguousarray(W_copy[0].reshape(DT, 128).T).astype(BF16)

    masked = np.zeros(VPAD, dtype=bool)
    masked[PAD_IDX] = True
    masked[VOCAB:] = True

    in_maps = []
    for c in range(NCORES):
        shard = Wq[c * VS:(c + 1) * VS]            # [VS, D] fp8
        wt_c = np.ascontiguousarray(
            shard.reshape(VS, QT, 2, 128).transpose(3, 2, 1, 0))
        mcount = int(masked[c * VS:(c + 1) * VS].sum())
        m = {
            "wt": wt_c,
            "htq": htq,
            "htb": htb,
            "attn_r": attn_r,
            "smap": smap,
            "wc": wc,
            "scol": scol,
            "mneg": np.array([[-float(mcount) / ATT]], dtype=np.float32),
        }
        if use_bgen:
            m["bg"] = bgp[c * VS:(c + 1) * VS].reshape(1, VS).astype(BF16)
        in_maps.append(m)

    res = run_bass_kernel_spmd(nc, in_maps, core_ids=list(range(NCORES)))
    LAST_RESULTS = res

    out = np.empty((N, VOCAB + CVOCAB), dtype=np.float32)
    for c in range(NCORES):
        lo = c * VS
        hi = min(lo + VS, VOCAB)
        if hi > lo:
            out[:, lo:hi] = res.results[c]["out_main"][:, :hi - lo].astype(
                np.float32)
    out[:, :VOCAB] *= np.float32(1.0 / OSCALE)
    out[:, PAD_IDX] = 0.0
    out[:, VOCAB:] = res.results[0]["out_copy"]
    return out


if __name__ == "__main__":
    # build-only smoke test
    nc = _get_nc(0.0, False)
    print("build OK:", nc)


# revision 26
# speedup vs baseline: 1.0377x; 1.0130x over previous
"""CopyGenerator on 8 TRN2 NeuronCores.

Strategy: tensor-parallel split of the 50257-wide generator vocab across the
8 cores (6400 padded columns each).  Per core:
  - W shard and hidden are quantized to fp8-e4m3 on host (W with one global
    scale, hidden with per-row scales) and fed to DoubleRow fp8 matmuls
    (2 k-subtiles per instruction -> 2x bf16 FLOPs on TRN2),
  - the combined dequant scale (s_row * s_W) is folded into the Exp
    activation's per-partition `scale` operand, so logits never materialize;
    a constant bias of -ln(32) keeps exp values inside fp8-e4m3 range,
  - exp results are stored fp8 in SBUF (25.6 KB/partition for 4 row tiles),
    with accum_out giving row partial sums,
  - softmax partial denominators are all-gathered once per FOUR row tiles
    ([128, 4] f32 - tiny, overlapped with the next group's matmuls),
  - exp is rescaled by (1 - p_copy)/denom on the DVE (fp8 in, bf16 out)
    and the output shard is written bf16 (host upcasts to f32),
  - copy gate z = h @ W_copy.T runs in bf16, copy-attention path in exact
    fp32, both redundantly on every core.  Gate/copy-path instructions are
    emitted between row tiles 1..3 so their late-arriving inputs never stall
    the in-order PE stream, and hidden/W DMAs are ordered so the first row
    tile's matmuls can start ~4us into the kernel.
PAD column and vocab-padding columns are handled by zeroing those W rows on
the host (=> logit 0, exp 1/32) and subtracting mcount/32 from the partial
denominator; the host zeroes the PAD output column.

kernel(**inputs) takes the full unsharded inputs and returns the full
[2048, 50321] float32 output.
"""

import os
import sys

for _p in ("/opt/trn_rl_repo", "/opt/trn_rl_repo/concourse"):
    if _p not in sys.path:
        sys.path.insert(0, _p)

from contextlib import ExitStack

import ml_dtypes
import numpy as np

import concourse.bass as bass
import concourse.mybir as mybir
import concourse.tile as tile
from concourse import bacc
from concourse.bass_utils import run_bass_kernel_spmd

# ---- problem constants (hardcoded per the self-contained-kernel contract) ----
N, D = 2048, 1024                 # tlen*batch rows, hidden dim
TLEN, BATCH, SLEN, CVOCAB = 64, 32, 128, 64
VOCAB = 50257
PAD_IDX = 0
NCORES = 8
VS = 6400                         # per-core padded vocab shard width
VPAD = VS * NCORES                # 51200
DT = D // 128                     # 8 contraction subtiles of 128
QT = DT // 2                      # 4 DoubleRow pairs
NT = N // 128                     # 16 row tiles
GRP = 4                           # row tiles per denominator collective
NG = NT // GRP
CH = [(0, 1536), (1536, 1536), (3072, 1536), (4608, 1536),
      (6144, 256)]                               # (offset, width) exp chunks
NCH = len(CH)
E4M3_MAX = 240.0
ATT = 32.0                        # exp attenuation keeping fp8 in range
BIAS = -float(np.log(ATT))
OSCALE = 4096.0                   # fp8 output scale (host divides it out)

E4 = ml_dtypes.float8_e4m3
BF16 = ml_dtypes.bfloat16
F32 = mybir.dt.float32
BF16_T = mybir.dt.bfloat16
FP8_T = mybir.dt.float8e4
DR = mybir.MatmulPerfMode.DoubleRow
DRS = mybir.MatmulPerfMode.DoubleRowSwInterleave

LAST_RESULTS = None               # BassKernelResults of the most recent run
_NC_CACHE = {}


def _build(bc_val: float, use_bgen: bool):
    nc = bacc.Bacc("TRN2", target_bir_lowering=False, debug=False,
                   num_devices=NCORES)

    wt = nc.dram_tensor("wt", [128, 2, QT, VS], FP8_T, kind="ExternalInput").ap()
    htq = nc.dram_tensor("htq", [128, NT, QT, 256], FP8_T,
                         kind="ExternalInput").ap()
    htb = nc.dram_tensor("htb", [128, DT, N], BF16_T, kind="ExternalInput").ap()
    attn_r = nc.dram_tensor("attn_r", [128, BATCH, TLEN], F32,
                            kind="ExternalInput").ap()
    smap = nc.dram_tensor("smap", [128, BATCH, CVOCAB], F32,
                          kind="ExternalInput").ap()
    wc = nc.dram_tensor("wc", [128, DT], BF16_T, kind="ExternalInput").ap()
    scol_d = nc.dram_tensor("scol", [128, NT], F32, kind="ExternalInput").ap()
    mneg = nc.dram_tensor("mneg", [1, 1], F32, kind="ExternalInput").ap()
    if use_bgen:
        bg = nc.dram_tensor("bg", [1, VS], BF16_T, kind="ExternalInput").ap()
    out_main = nc.dram_tensor("out_main", [N, VS], FP8_T,
                              kind="ExternalOutput").ap()
    out_copy = nc.dram_tensor("out_copy", [N, CVOCAB], F32,
                              kind="ExternalOutput").ap()

    with tile.TileContext(nc) as tc, ExitStack() as ctx:
        singles = ctx.enter_context(tc.tile_pool(name="singles", bufs=1))
        dram = ctx.enter_context(tc.tile_pool(name="dram", bufs=1, space="DRAM"))

        # ---- resident inputs; order = DMA issue order (first needed first,
        # tiny consumers of the first exps before the bulk weights) ----
        htq_sb = singles.tile([128, NT, QT, 256], FP8_T)
        nc.sync.dma_start(out=htq_sb[:, 0, :, :], in_=htq[:, 0, :, :])
        wt_sb = singles.tile([128, 2, QT, VS], FP8_T)
        nc.sync.dma_start(out=wt_sb[:, :, :, 0:512], in_=wt[:, :, :, 0:512])
        scol_sb = singles.tile([128, NT], F32)
        nc.sync.dma_start(out=scol_sb, in_=scol_d)
        wc_sb = singles.tile([128, DT], BF16_T)
        nc.sync.dma_start(out=wc_sb, in_=wc)
        mneg_sb = singles.tile([128, 1], F32)
        nc.gpsimd.dma_start(out=mneg_sb, in_=mneg.to_broadcast((128, 1)))
        bias_sb = singles.tile([128, 1], F32)
        nc.vector.memset(bias_sb, BIAS)
        for c0 in range(512, 2048, 512):
            nc.sync.dma_start(out=wt_sb[:, :, :, c0:c0 + 512],
                              in_=wt[:, :, :, c0:c0 + 512])
        nc.sync.dma_start(out=htq_sb[:, 1, :, :], in_=htq[:, 1, :, :])
        for c0 in range(2048, VS, 1024):
            cw = min(1024, VS - c0)
            nc.sync.dma_start(out=wt_sb[:, :, :, c0:c0 + cw],
                              in_=wt[:, :, :, c0:c0 + cw])
        for j in (2, 3):
            nc.sync.dma_start(out=htq_sb[:, j, :, :], in_=htq[:, j, :, :])
        htb_sb = singles.tile([128, DT, N], BF16_T)
        nc.sync.dma_start(out=htb_sb, in_=htb)
        attn_sb = singles.tile([128, BATCH, TLEN], F32)
        nc.sync.dma_start(out=attn_sb, in_=attn_r)
        sm_sb = singles.tile([128, BATCH, CVOCAB], F32)
        nc.sync.dma_start(out=sm_sb, in_=smap)
        nc.sync.dma_start(out=htq_sb[:, 4:NT, :, :], in_=htq[:, 4:NT, :, :])
        if use_bgen:
            bg_sb = singles.tile([1, VS], BF16_T)
            nc.sync.dma_start(out=bg_sb, in_=bg)
            ones_sb = singles.tile([1, N], BF16_T)
            nc.vector.memset(ones_sb, 1.0)

        zcol = singles.tile([128, NT], F32)
        ompcol = singles.tile([128, NT], F32)   # 1 - p_copy = sigmoid(-z - bc)

        cps = ctx.enter_context(tc.tile_pool(name="cps", bufs=1))
        ocp = ctx.enter_context(tc.tile_pool(name="ocp", bufs=2))
        expp = ctx.enter_context(tc.tile_pool(name="expp", bufs=8))
        accp = ctx.enter_context(tc.tile_pool(name="accp", bufs=4))
        small = ctx.enter_context(tc.tile_pool(name="small", bufs=4))
        ostp = ctx.enter_context(tc.tile_pool(name="ostp", bufs=2))
        ps_cp = ctx.enter_context(
            tc.tile_pool(name="ps_cp", bufs=1, space="PSUM"))
        ps_256 = ctx.enter_context(
            tc.tile_pool(name="ps_256", bufs=1, space="PSUM"))
        ps_main = ctx.enter_context(
            tc.tile_pool(name="ps_main", bufs=2, space="PSUM"))

        def emit_gate():
            # ---- copy-gate z = hidden @ W_copy.T  (M=1 bf16 matmuls) ----
            z_sb = cps.tile([1, N], F32)
            for q in range(N // 512):
                zp = ps_cp.tile([1, 512], F32, tag="cp",
                                padded_shape=[TLEN, 8 * CVOCAB])
                for d in range(DT):
                    nc.tensor.matmul(
                        zp,
                        lhsT=wc_sb[:, d:d + 1],
                        rhs=htb_sb[:, d, q * 512:(q + 1) * 512],
                        start=(d == 0), stop=(d == DT - 1),
                    )
                nc.scalar.copy(out=z_sb[:, q * 512:(q + 1) * 512], in_=zp)
            zdram = dram.tile([N], F32)
            nc.sync.dma_start(out=zdram.rearrange("(a n) -> a n", a=1), in_=z_sb)
            # per-row-tile column layout [128, 16] and per-(t,b) layout [64, 32]
            nc.scalar.dma_start(out=zcol,
                                in_=zdram.rearrange("(j p) -> p j", p=128))
            zbt = cps.tile([TLEN, BATCH], F32)
            nc.scalar.dma_start(out=zbt,
                                in_=zdram.rearrange("(t b) -> t b", b=BATCH))
            nc.scalar.activation(ompcol, zcol,
                                 mybir.ActivationFunctionType.Sigmoid,
                                 bias=-bc_val, scale=-1.0)
            pcbt = cps.tile([TLEN, BATCH], F32)  # p_copy = sigmoid(z + bc)
            nc.scalar.activation(pcbt, zbt,
                                 mybir.ActivationFunctionType.Sigmoid,
                                 bias=bc_val, scale=1.0)
            return pcbt

        def emit_copy_path(pcbt):
            # ---- copy path: per-batch [64t,128s] @ [128s,64c] x p_copy,
            # 8 batches share one PSUM bank / output DMA so the PE stream
            # never waits on single-batch drains ----
            oc3 = out_copy.rearrange("(t b) c -> t (b c)", b=BATCH)
            for g in range(BATCH // 8):
                cp = ps_cp.tile([TLEN, 8 * CVOCAB], F32, tag="cp")
                for bi in range(8):
                    b = g * 8 + bi
                    nc.tensor.matmul(
                        cp[:, bi * CVOCAB:(bi + 1) * CVOCAB],
                        lhsT=attn_sb[:, b, :],
                        rhs=sm_sb[:, b, :],
                        start=True, stop=True,
                    )
                oc = ocp.tile([TLEN, 8 * CVOCAB], F32, tag="oc")
                for bi in range(8):
                    b = g * 8 + bi
                    nc.vector.tensor_scalar_mul(
                        oc[:, bi * CVOCAB:(bi + 1) * CVOCAB],
                        cp[:, bi * CVOCAB:(bi + 1) * CVOCAB],
                        pcbt[:, b:b + 1])
                nc.sync.dma_start(
                    out=oc3[:, g * 8 * CVOCAB:(g + 1) * 8 * CVOCAB], in_=oc)

        # ---- main loop: one denominator collective per 4 row tiles ----
        pcbt = None
        exp_tiles = {}
        for g0, gsz in [(0, 2), (2, 4), (6, 4), (10, 4), (14, 2)]:
            ccst = small.tile([128, gsz], F32, tag=f"ccst{gsz}")
            for t in range(gsz):
                j = g0 + t
                exp_sb = expp.tile([128, VS], FP8_T, tag="exp")
                exp_tiles[j] = exp_sb
                acc7 = accp.tile([128, NCH], F32, tag="acc7")
                for ch in range(NCH):
                    c0, cw = CH[ch]
                    if cw == 256:
                        psm = ps_256.tile([128, 256], F32, tag="p256")
                    else:
                        psm = ps_main.tile([128, cw], F32, tag="psm",
                                           padded_shape=[128, 1536])
                    for h0 in range(0, cw, 512):
                        hw = min(512, cw - h0)
                        for q in range(QT):
                            nc.tensor.matmul(
                                psm[:, h0:h0 + hw],
                                lhsT=htq_sb[:, j, q, :],
                                rhs=wt_sb[:, :, q, c0 + h0:c0 + h0 + hw],
                                start=(q == 0),
                                stop=(q == QT - 1) and not use_bgen,
                                perf_mode=DRS,
                            )
                        if use_bgen:
                            nc.tensor.matmul(
                                psm[:, h0:h0 + hw],
                                lhsT=ones_sb[:, j * 128:(j + 1) * 128],
                                rhs=bg_sb[:, c0 + h0:c0 + h0 + hw],
                                start=False, stop=True,
                            )
                    nc.scalar.activation(exp_sb[:, c0:c0 + cw],
                                         psm,
                                         mybir.ActivationFunctionType.Exp,
                                         bias=bias_sb,
                                         scale=scol_sb[:, j:j + 1],
                                         accum_out=acc7[:, ch:ch + 1])
                accsum = small.tile([128, 1], F32, tag="accsum")
                nc.vector.reduce_sum(accsum, acc7, axis=mybir.AxisListType.X)
                nc.vector.tensor_scalar_add(ccst[:, t:t + 1], accsum, mneg_sb)
                if j == 1:
                    pcbt = emit_gate()
                elif j == 4:
                    emit_copy_path(pcbt)
            ccin = dram.tile([128, gsz], F32, tag=f"ccin{gsz}", bufs=2)
            nc.sync.dma_start(out=ccin, in_=ccst)
            ccout = dram.tile([NCORES * 128 * gsz], F32, tag=f"ccout{gsz}",
                              bufs=2)
            nc.gpsimd.collective_compute(
                "AllGather", mybir.AluOpType.bypass,
                replica_groups=[list(range(NCORES))],
                ins=[ccin.opt()], outs=[ccout.opt()],
            )
            parts = small.tile([128, gsz, NCORES], F32, tag=f"parts{gsz}")
            nc.sync.dma_start(
                out=parts,
                in_=ccout.rearrange("(r p t) -> p t r", p=128, t=gsz))
            last = g0 + gsz == NT
            fss = {}
            if last:
                for t in range(gsz):
                    j = g0 + t
                    denom = small.tile([128, 1], F32, tag="denom")
                    nc.vector.reduce_sum(denom, parts[:, t, :],
                                         axis=mybir.AxisListType.X)
                    rden = small.tile([128, 1], F32, tag="rden")
                    nc.vector.reciprocal(rden, denom)
                    fs = small.tile([128, 1], F32, tag=f"fs{t}")
                    nc.vector.tensor_scalar(fs, rden, ompcol[:, j:j + 1],
                                            OSCALE, mybir.AluOpType.mult,
                                            mybir.AluOpType.mult)
                    fss[j] = fs
            for t in range(gsz):
                j = g0 + t
                n0 = j * 128
                if last:
                    fs = fss[j]
                else:
                    denom = small.tile([128, 1], F32, tag="denom")
                    nc.vector.reduce_sum(denom, parts[:, t, :],
                                         axis=mybir.AxisListType.X)
                    rden = small.tile([128, 1], F32, tag="rden")
                    nc.vector.reciprocal(rden, denom)
                    fs = small.tile([128, 1], F32, tag="fs")
                    nc.vector.tensor_scalar(fs, rden, ompcol[:, j:j + 1],
                                            OSCALE, mybir.AluOpType.mult,
                                            mybir.AluOpType.mult)
                ost = ostp.tile([128, VS], FP8_T, tag="ost")
                if last:
                    # drain the final group on both element-wise engines
                    # (2/3 DVE, 1/3 Act Copy) so the tail is DMA-bound
                    HV = 4352
                    nc.vector.tensor_scalar_mul(ost[:, 0:HV],
                                                exp_tiles[j][:, 0:HV], fs)
                    nc.sync.dma_start(out=out_main[n0:n0 + 128, 0:HV],
                                      in_=ost[:, 0:HV])
                    nc.scalar.activation(ost[:, HV:VS], exp_tiles[j][:, HV:VS],
                                         mybir.ActivationFunctionType.Copy,
                                         scale=fs)
                    nc.sync.dma_start(out=out_main[n0:n0 + 128, HV:VS],
                                      in_=ost[:, HV:VS])
                else:
                    nc.vector.tensor_scalar_mul(ost, exp_tiles[j], fs)
                    nc.sync.dma_start(out=out_main[n0:n0 + 128, :], in_=ost)

    nc.compile()
    return nc


def _get_nc(bc_val: float, use_bgen: bool):
    key = (bc_val, use_bgen)
    if key not in _NC_CACHE:
        _NC_CACHE[key] = _build(bc_val, use_bgen)
    return _NC_CACHE[key]


def kernel(hidden, attn, src_map, W_gen, b_gen, W_copy, b_copy):
    global LAST_RESULTS
    hidden = np.asarray(hidden, dtype=np.float32)
    attn = np.asarray(attn, dtype=np.float32)
    src_map = np.asarray(src_map, dtype=np.float32)
    W_gen = np.asarray(W_gen, dtype=np.float32)
    b_gen = np.asarray(b_gen, dtype=np.float32)
    W_copy = np.asarray(W_copy, dtype=np.float32)
    b_copy = np.asarray(b_copy, dtype=np.float32)

    use_bgen = bool(np.any(b_gen))
    bc_val = float(b_copy.reshape(-1)[0])
    nc = _get_nc(bc_val, use_bgen)

    # per-row fp8 scales for hidden; one global scale for W
    s_n = np.abs(hidden).max(axis=1, keepdims=True) / E4M3_MAX   # [N, 1]
    s_n = np.maximum(s_n, 1e-30)
    hq = (hidden / s_n).astype(E4)
    s_w = max(float(np.abs(W_gen).max()) / E4M3_MAX, 1e-30)

    # hq tiled for DoubleRowSwInterleave, row-tile-major; per (j, q) the
    # [128, 256] stationary block holds (A,B) k-subtile pairs interleaved
    # per column with columns reversed: swi[p, 2k+i] = hq[j*128+127-k,
    # q*256 + i*128 + p]
    htq = hq.reshape(NT, 128, QT, 2, 128).transpose(4, 0, 2, 1, 3)
    htq = np.ascontiguousarray(htq[:, :, :, ::-1, :]).reshape(
        128, NT, QT, 256)
    # bf16 hidden for the copy gate: htb[p, d, n] = h[n, d*128 + p]
    htb = np.ascontiguousarray(
        hidden.reshape(N, DT, 128).transpose(2, 1, 0)).astype(BF16)

    # combined dequant scale per row, tiled [128, NT]
    scol = np.ascontiguousarray(
        (s_n[:, 0] * s_w).astype(np.float32).reshape(NT, 128).T)

    # padded W with masked rows zeroed (PAD row + vocab padding)
    Wp = np.zeros((VPAD, D), dtype=np.float32)
    Wp[:VOCAB] = W_gen
    Wp[PAD_IDX] = 0.0
    Wq = (Wp / s_w).astype(E4)
    Wq[PAD_IDX] = 0
    Wq[VOCAB:] = 0
    if use_bgen:
        bgp = np.zeros((VPAD,), dtype=np.float32)
        bgp[:VOCAB] = b_gen
        bgp[PAD_IDX] = 0.0

    # attn rearranged to [s, b, t] f32; smap [s, b, c] f32
    attn_r = np.ascontiguousarray(
        attn.reshape(TLEN, BATCH, SLEN).transpose(2, 1, 0))
    smap = np.ascontiguousarray(src_map.astype(np.float32))
    wc = np.ascontiguousarray(W_copy[0].reshape(DT, 128).T).astype(BF16)

    masked = np.zeros(VPAD, dtype=bool)
    masked[PAD_IDX] = True
    masked[VOCAB:] = True

    in_maps = []
    for c in range(NCORES):
        shard = Wq[c * VS:(c + 1) * VS]            # [VS, D] fp8
        wt_c = np.ascontiguousarray(
            shard.reshape(VS, QT, 2, 128).transpose(3, 2, 1, 0))
        mcount = int(masked[c * VS:(c + 1) * VS].sum())
        m = {
            "wt": wt_c,
            "htq": htq,
            "htb": htb,
            "attn_r": attn_r,
            "smap": smap,
            "wc": wc,
            "scol": scol,
            "mneg": np.array([[-float(mcount) / ATT]], dtype=np.float32),
        }
        if use_bgen:
            m["bg"] = bgp[c * VS:(c + 1) * VS].reshape(1, VS).astype(BF16)
        in_maps.append(m)

    res = run_bass_kernel_spmd(nc, in_maps, core_ids=list(range(NCORES)))
    LAST_RESULTS = res

    out = np.empty((N, VOCAB + CVOCAB), dtype=np.float32)
    for c in range(NCORES):
        lo = c * VS
        hi = min(lo + VS, VOCAB)
        if hi > lo:
            out[:, lo:hi] = res.results[c]["out_main"][:, :hi - lo].astype(
                np.float32)
    out[:, :VOCAB] *= np.float32(1.0 / OSCALE)
    out[:, PAD_IDX] = 0.0
    out[:, VOCAB:] = res.results[0]["out_copy"]
    return out


if __name__ == "__main__":
    # build-only smoke test
    nc = _get_nc(0.0, False)
    print("build OK:", nc)
